# revision 2
# baseline (speedup 1.0000x reference)
"""Trainium2 Bass kernel v6 for nn_BaseTransformer (ensemble member-attention).

Data-parallel over batch B=8 across 8 cores. bf16 IO (host casts x down,
output back up). Per core, x = in_tensor[b] as [K=16, C=64, S=4096]:

  kq   = selu-ish: kqT stores selu(k)/lambda etc (lambda^2 folded in GRAM_SCALE)
  gram[c,i,j] = sum_s kq..., softmax over i (mask-then-reduce), B = E_norm + D
  mix per head-group g (8 heads, stride-8: c = 8u+g)
  out = selu(x + wout.T transformed + bout)    (lambda folds in weights)

Layouts (i = member = 2t + m2, pair tile t holds members 2t, 2t+1):
  xb[t]   [128=(m2,c'), S]                       resident bf16
  kqT blk [128=s, 2048], col = 1024*half + 16*h + i     (h = head, natural)
  vT blk  [128=s, 1024], col = 128*g + 16*u + i         (head c = 8u+g)
  gram_ps [p = 16u + j, f = 16u' + i] per g   (mask kills u!=u')
  rho_v(i,u) = 16u + i ; rho_T(j,u) = 16*(j//2) + 8*(j%2) + u
  vg blk  [128=rho_v, 1024], col = 128*g + s_in          (via XBAR)
  TT blk  [128=s, 1024], col = 128*jt + rho_T            (mix out copy)
  T_pair  [128=(16g+8j2+u), 4096], col = 512*jt + 128*blkw + s_in  (via XBAR)
  out psum rows = 64*j2 + o  == xb pair rows  -> lamI residual matmul
"""

import sys

if "/opt/trn_rl_repo" not in sys.path:
    sys.path.insert(0, "/opt/trn_rl_repo")

import numpy as np
import ml_dtypes

import concourse.bass as bass
import concourse.bacc as bacc
import concourse.mybir as mybir
import concourse.tile as tile

F32 = mybir.dt.float32
BF16 = mybir.dt.bfloat16
BF = ml_dtypes.bfloat16

K, C, S = 16, 64, 4096
NB = 32          # 128-col spatial blocks
SC = 128

ALPHA = 1.6732632423543772
LAMBDA = 1.0507009873554805
LN_ALPHA = float(np.log(ALPHA))
LN_LAMBDA_ALPHA = float(np.log(LAMBDA * ALPHA))
GRAM_SCALE = float(LAMBDA * LAMBDA / 64.0)


def host_constants(w_value, w_key, w_query, w_out, b_out):
    """Replicated device inputs (host-side weight shuffles + folds)."""
    c = {}
    # wkqv2 [128, 384]: rows (m2, c'); cols 192*m2 + [k 0:64 | q 64:128 | v 128:192]
    # k/q cols: head h natural. v cols: position 8*(h%8) + (h//8)  (= 8g+u)
    # cols: [k(m2=0) | k(m2=1) | q(m2=0) | q(m2=1) | v(m2=0) | v(m2=1)]
    # kq head order natural; v head at position 8*(h%8) + h//8 (= 8g+u)
    wk, wq, wv = w_key, w_query, w_value          # [heads, c_in]
    vperm = np.zeros(64, np.int64)
    for h in range(64):
        vperm[8 * (h % 8) + (h // 8)] = h          # col pos 8g+u <- head h=8u+g
    vv = wv.T[:, vperm]                             # [c', 64]
    blk = np.zeros((128, 384), np.float32)
    for m2 in range(2):
        r = slice(64 * m2, 64 * m2 + 64)
        blk[r, 64 * m2:64 * m2 + 64] = wk.T
        blk[r, 128 + 64 * m2:128 + 64 * m2 + 64] = wq.T
        blk[r, 256 + 64 * m2:256 + 64 * m2 + 64] = vv
    c["wkqv2"] = blk

    # wout2 [128, 128]: rows 16g + 8j2 + u ; cols 64*j2 + o ; val = lam*wout[o, 8u+g]
    w2 = np.zeros((128, 128), np.float32)
    for g in range(8):
        for j2 in range(2):
            for u in range(8):
                w2[16 * g + 8 * j2 + u, 64 * j2:64 * j2 + 64] = \
                    LAMBDA * w_out[:, 8 * u + g]
    c["wout2"] = w2

    c["lamI"] = (LAMBDA * np.eye(128)).astype(np.float32)

    # mask on gram [p=8j+u, f=8i+u']: keep u==u'
    mask = np.zeros((128, 128), np.float32)
    for p in range(128):
        for f in range(128):
            if p % 8 == f % 8:
                mask[p, f] = 1.0
    c["maskg"] = mask

    # Pp2 [k = 8j+u, n = rho_T(j,u)]
    pp = np.zeros((128, 128), np.float32)
    for u in range(8):
        for j in range(16):
            pp[8 * j + u, 16 * (j // 2) + 8 * (j % 2) + u] = 1.0
    c["permPp"] = pp

    # P2 [k = 8i+u, m = rho_v(i,u) = 16u+i]  (second perm lhsT)
    p2 = np.zeros((128, 128), np.float32)
    for u in range(8):
        for i in range(16):
            p2[8 * i + u, 16 * u + i] = 1.0
    c["permP2"] = p2

    # D2 [rho_v(i,u), rho_T(j,u')] = delta(u,u') (delta(i,j) - 1/16)
    d2 = np.zeros((128, 128), np.float32)
    for u in range(8):
        for i in range(16):
            for j in range(16):
                d2[16 * u + i, 16 * (j // 2) + 8 * (j % 2) + u] = \
                    (1.0 if i == j else 0.0) - 1.0 / 16.0
    c["dpat2"] = d2

    # exp2 bias: rows (j2, o): bout[o] + ln(lam*alpha);  relu2 bias: lam*bout[o]
    b1 = np.concatenate([b_out, b_out]).astype(np.float32) + LN_LAMBDA_ALPHA
    c["bias_e"] = b1.reshape(128, 1)
    b2 = LAMBDA * np.concatenate([b_out, b_out]).astype(np.float32)
    c["bias_r"] = b2.reshape(128, 1)
    return {k2: np.ascontiguousarray(v) for k2, v in c.items()}


def build_nc():
    nc = bacc.Bacc("TRN2", target_bir_lowering=False, debug=False)

    x_d = nc.dram_tensor("x", [K, C, S], BF16, kind="ExternalInput")
    wkqv_d = nc.dram_tensor("wkqv2", [128, 384], F32, kind="ExternalInput")
    wout_d = nc.dram_tensor("wout2", [128, 128], F32, kind="ExternalInput")
    lamI_d = nc.dram_tensor("lamI", [128, 128], F32, kind="ExternalInput")
    mask_d = nc.dram_tensor("maskg", [128, 128], F32, kind="ExternalInput")
    pp_d = nc.dram_tensor("permPp", [128, 128], F32, kind="ExternalInput")
    p2_d = nc.dram_tensor("permP2", [128, 128], F32, kind="ExternalInput")
    d2_d = nc.dram_tensor("dpat2", [128, 128], F32, kind="ExternalInput")
    be_d = nc.dram_tensor("bias_e", [128, 1], F32, kind="ExternalInput")
    br_d = nc.dram_tensor("bias_r", [128, 1], F32, kind="ExternalInput")
    out_d = nc.dram_tensor("out", [K, C, S], BF16, kind="ExternalOutput")

    with tile.TileContext(nc) as tc:
        with (
            tc.tile_pool(name="persist", bufs=1) as persist,
            tc.tile_pool(name="xpool", bufs=1) as xpool,
            tc.tile_pool(name="vtpool", bufs=1) as vtpool,
        ):
            # ---------- constants ----------
            def load_const(name, dram, shape, cast_bf16=False):
                t = persist.tile(shape, F32, tag=name + "f")
                nc.sync.dma_start(out=t, in_=dram[:, :])
                if not cast_bf16:
                    return t
                tb = persist.tile(shape, BF16, tag=name)
                nc.gpsimd.tensor_copy(tb, t)
                return tb

            wkqv_sb = load_const("wkqv", wkqv_d, [128, 384], cast_bf16=True)
            wout_sb = load_const("wout", wout_d, [128, 128], cast_bf16=True)
            lamI_sb = load_const("lamI", lamI_d, [128, 128], cast_bf16=True)
            mask_sb = load_const("mask", mask_d, [128, 128])
            pp_sb = load_const("pp", pp_d, [128, 128], cast_bf16=True)
            p2_sb = load_const("p2", p2_d, [128, 128], cast_bf16=True)
            d2_sb = load_const("d2", d2_d, [128, 128])
            be_sb = load_const("be", be_d, [128, 1])
            br_sb = load_const("br", br_d, [128, 1])
            lna_sb = persist.tile([128, 1], F32, tag="lna")
            nc.vector.memset(lna_sb, LN_ALPHA)

            # ---------- x resident: xb[t] = members (2t, 2t+1) ----------
            x2 = x_d.rearrange("(t m2) c s -> t (m2 c) s", t=8, m2=2)
            x_sb = []
            for t in range(8):
                xb = xpool.tile([128, S], BF16, tag=f"x{t}")
                nc.sync.dma_start(out=xb, in_=x2[t])
                x_sb.append(xb)

            # vT blocks resident until consumed by XBAR in ph2 (freed never;
            # one [128, 1024] per block -> 32 * 2KB/part = 64KB/part)
            vt_sb = []
            for b in range(NB):
                vt_sb.append(vtpool.tile([128, 1024], BF16, tag=f"vt{b}",
                                         name=f"vt{b}"))

            bigB = [persist.tile([128, 128], BF16, tag=f"bigB{g}",
                                 name=f"bigB{g}") for g in range(8)]

            # ======================= PHASE 1 =======================
            with (
                tc.tile_pool(name="p1sc", bufs=3) as p1sc,
                tc.tile_pool(name="gramps", bufs=1, space="PSUM") as gramps,
            ):
                gram_ps = [gramps.tile([128, 512], F32, tag=f"gram{gb}",
                                       name=f"gram{gb}") for gb in range(2)]

                kqvps_cm = tc.tile_pool(name="kqvps", bufs=3, space="PSUM")
                kqvps = kqvps_cm.__enter__()
                kqT_hist = {}
                for blk in range(NB):
                    sl = slice(SC * blk, SC * (blk + 1))
                    e2 = p1sc.tile([128, 2048], BF16, tag="e2")
                    r2 = p1sc.tile([128, 2048], BF16, tag="r2")
                    for (t0, nt) in ((0, 2), (2, 2), (4, 2), (6, 2)):
                        ps = kqvps.tile([128, 1024], F32, tag="kqv")
                        for tb in range(nt):
                            nc.tensor.matmul(
                                ps[:, 512 * tb: 512 * tb + 384],
                                x_sb[t0 + tb][:, sl], wkqv_sb,
                                start=True, stop=True)
                        # psum cols per pair: [k0 k1 q0 q1 | v0 v1]
                        kq_src = ps.rearrange(
                            "p (tb r) -> p tb r", tb=2, r=512
                        )[:, 0:nt, 0:256].rearrange(
                            "p tb (half mh) -> p tb half mh", half=2, mh=128)
                        # dst kqT-layout col = 1024*half + 128*t + mh
                        def shaped(dst):
                            v = dst.rearrange(
                                "p (half t mh) -> p t half mh",
                                half=2, t=8, mh=128)
                            return v[:, t0:t0 + nt]     # [p, tb, half, mh]
                        nc.scalar.activation(
                            out=shaped(e2), in_=kq_src,
                            func=mybir.ActivationFunctionType.Exp,
                            bias=lna_sb[:, 0:1])
                        nc.vector.tensor_scalar(
                            out=shaped(r2), in0=kq_src,
                            scalar1=0.0, scalar2=ALPHA,
                            op0=mybir.AluOpType.max,
                            op1=mybir.AluOpType.add)
                        # v cols [256:384]: (tb, m2: 64, hv: 1 x64)
                        # dst vT col = 16*hv + i,  i = 2t + m2
                        v_src = ps.rearrange(
                            "p (tb r) -> p tb r", tb=2, r=512
                        )[:, 0:nt, 256:384].rearrange(
                            "p tb (m2 hv) -> p tb m2 hv", m2=2, hv=64)
                        v_dst = vt_sb[blk].rearrange(
                            "p (hv i) -> p hv i", hv=64, i=16).rearrange(
                            "p hv (t m2) -> p t m2 hv",
                            t=8, m2=2)[:, t0:t0 + nt]
                        if t0 < 4:
                            nc.scalar.copy(v_dst, v_src)
                        else:
                            nc.vector.tensor_copy(v_dst, v_src)
                    # mq = min(e2, relu+a) [Pool]; kqT = mq - a = selu/lam [4x]
                    mq = p1sc.tile([128, 2048], BF16, tag="mq")
                    nc.vector.tensor_tensor(
                        out=mq, in0=e2, in1=r2, op=mybir.AluOpType.min)
                    kqT = p1sc.tile([128, 2048], BF16, tag="kqT")
                    nc.vector.tensor_scalar(
                        out=kqT, in0=mq, scalar1=ALPHA, scalar2=None,
                        op0=mybir.AluOpType.subtract)
                    # gram software-pipelined one block behind (PE never
                    # waits on this block's selu chain)
                    kqT_hist[blk] = kqT
                    def emit_gram(b):
                        kv = kqT_hist.pop(b).rearrange(
                            "p (f e) -> p e f", f=256, e=8)
                        for g in range(8):
                            nc.tensor.matmul(
                                gram_ps[g // 4][:, 128 * (g % 4):
                                                128 * (g % 4 + 1)],
                                kv[:, g, 128:256], kv[:, g, 0:128],
                                start=(b == 0), stop=(b == NB - 1))
                    if blk > 0:
                        emit_gram(blk - 1)
                    if blk == NB - 1:
                        emit_gram(blk)

                # ---------- softmax + bigB ----------
                kqvps_cm.__exit__(None, None, None)
                with tc.tile_pool(name="smps", bufs=1, space="PSUM") as smps:
                    for g in range(8):
                        gp = gram_ps[g // 4][:, 128 * (g % 4): 128 * (g % 4 + 1)]
                        E = p1sc.tile([128, 128], F32, tag="E")
                        nc.scalar.activation(
                            out=E, in_=gp,
                            func=mybir.ActivationFunctionType.Exp,
                            bias=0.0, scale=GRAM_SCALE)
                        Em = p1sc.tile([128, 128], F32, tag="Em")
                        nc.vector.tensor_tensor(
                            out=Em, in0=E, in1=mask_sb,
                            op=mybir.AluOpType.mult)
                        Ss = p1sc.tile([128, 1], F32, tag="Ss")
                        nc.vector.tensor_reduce(
                            out=Ss, in_=Em, axis=mybir.AxisListType.X,
                            op=mybir.AluOpType.add)
                        R2 = p1sc.tile([128, 1], F32, tag="R2")
                        nc.vector.reciprocal(out=R2, in_=Ss)
                        En = p1sc.tile([128, 128], BF16, tag="En")
                        nc.vector.tensor_scalar(
                            out=En, in0=Em, scalar1=R2[:, 0:1], scalar2=None,
                            op0=mybir.AluOpType.mult)
                        # c[8i+u, rho_T(j,u)] = En[8j+u, 8i+u]
                        cps = smps.tile([128, 128], F32, tag="cps")
                        nc.tensor.matmul(cps, En, pp_sb, start=True, stop=True)
                        csb = p1sc.tile([128, 128], BF16, tag="csb")
                        nc.vector.tensor_copy(csb, cps)
                        # b[rho_v(i,u), rho_T(j,u)] = c[8i+u, .] ; + D2
                        bps = smps.tile([128, 128], F32, tag="bps")
                        nc.tensor.matmul(bps, p2_sb, csb, start=True, stop=True)
                        nc.vector.scalar_tensor_tensor(
                            out=bigB[g], in0=bps, scalar=1.0, in1=d2_sb,
                            op0=mybir.AluOpType.mult, op1=mybir.AluOpType.add)

            # ======================= PHASE 2 =======================
            with (
                tc.tile_pool(name="vgp", bufs=4) as vgp,
                tc.tile_pool(name="ttp", bufs=4) as ttp,
                tc.tile_pool(name="tpp", bufs=2) as tpp,
                tc.tile_pool(name="osb", bufs=2) as osbp,
                tc.tile_pool(name="p2sc", bufs=3) as p2sc,
                tc.tile_pool(name="mixps", bufs=2, space="PSUM") as mixps,
                tc.tile_pool(name="ops", bufs=2, space="PSUM") as ops,
            ):
                ov_d = out_d.rearrange(
                    "(jt j2) c (ch s) -> ch (j2 c) jt s", jt=8, j2=2, ch=8)
                tp_hist = {}

                def emit_out(ch):
                    tp = tp_hist.pop(ch)
                    slc = slice(512 * ch, 512 * (ch + 1))
                    ost = osbp.tile([128, 4096], BF16, tag="ost")
                    for th in range(4):
                        po = ops.tile([128, 1024], F32, tag="ops")
                        for sub in range(2):
                            jt = 2 * th + sub
                            pslice = po[:, 512 * sub: 512 * (sub + 1)]
                            nc.tensor.matmul(
                                pslice, wout_sb,
                                tp[:, 512 * jt: 512 * (jt + 1)],
                                start=True, stop=False)
                            nc.tensor.matmul(
                                pslice, lamI_sb, x_sb[jt][:, slc],
                                start=False, stop=True)
                        e2o = p2sc.tile([128, 1024], BF16, tag="e2o")
                        nc.scalar.activation(
                            out=e2o, in_=po,
                            func=mybir.ActivationFunctionType.Exp,
                            bias=be_sb[:, 0:1], scale=float(1.0 / LAMBDA))
                        r3 = p2sc.tile([128, 1024], BF16, tag="r3")
                        nc.scalar.activation(
                            out=r3, in_=po,
                            func=mybir.ActivationFunctionType.Relu,
                            bias=br_sb[:, 0:1])
                        e2m = p2sc.tile([128, 1024], BF16, tag="e2m")
                        nc.vector.tensor_scalar(
                            out=e2m, in0=e2o,
                            scalar1=float(LAMBDA * ALPHA), scalar2=None,
                            op0=mybir.AluOpType.subtract)
                        nc.vector.tensor_tensor(
                            out=ost[:, 1024 * th: 1024 * (th + 1)],
                            in0=e2m, in1=r3, op=mybir.AluOpType.min)
                    ov = ost.rearrange("p (jt s) -> p jt s", jt=8, s=512)
                    nc.sync.dma_start(out=ov_d[ch], in_=ov)

                for ch in range(8):             # 512-col chunks
                    tp = tpp.tile([128, 4096], BF16, tag="tpair")
                    tp_hist[ch] = tp
                    for blkw in range(4):
                        blk = 4 * ch + blkw
                        # v regroup: vT [s, (g, rho_v)] -> vg [rho_v, (g, s)]
                        vg = vgp.tile([128, 1024], BF16, tag="vg")
                        vgv = vg.rearrange("p (g s) -> p g s", g=8, s=128)
                        nc.sync.dma_start(out=vgv, in_=vt_sb[blk],
                                          transpose=True)
                        # mix (v stationary): psum [s, rho_T] per g
                        mps = mixps.tile([128, 1024], F32, tag="mix")
                        for g in range(8):
                            nc.tensor.matmul(
                                mps[:, 128 * g: 128 * (g + 1)],
                                vgv[:, g], bigB[g], start=True, stop=True)
                        # copy -> TT col = 128*jt + 16*g + r  (rho_T=(jt,j2,u))
                        tt = ttp.tile([128, 1024], BF16, tag="tt")
                        tt_dst = tt.rearrange(
                            "p (jt g r) -> p g jt r", jt=8, g=8, r=16)
                        mps_src = mps.rearrange(
                            "p (g jt r) -> p g jt r", g=8, jt=8, r=16)
                        nc.vector.tensor_copy(tt_dst, mps_src)
                        # T regroup: TT [s, (jt, rho)] -> T_pair [rho, (jt, s)]
                        tpv = tp.rearrange("p (jt w s) -> p jt w s",
                                           jt=8, w=4, s=128)[:, :, blkw]
                        nc.sync.dma_start(out=tpv, in_=tt, transpose=True)
                    if ch > 0:
                        emit_out(ch - 1)
                    if ch == 7:
                        emit_out(ch)
    nc.compile()
    return nc


_NC_CACHE = None


def _get_nc():
    global _NC_CACHE
    if _NC_CACHE is None:
        _NC_CACHE = build_nc()
    return _NC_CACHE


def kernel(in_tensor, w_value, w_key, w_query, w_out, b_out, **_ignored):
    in_tensor = np.asarray(in_tensor, dtype=np.float32)
    consts = host_constants(
        np.asarray(w_value, np.float32), np.asarray(w_key, np.float32),
        np.asarray(w_query, np.float32), np.asarray(w_out, np.float32),
        np.asarray(b_out, np.float32))

    B = in_tensor.shape[0]
    assert B == 8
    nc = _get_nc()
    in_maps = []
    for b in range(B):
        m = {"x": np.ascontiguousarray(
            in_tensor[b].reshape(K, C, S)).astype(BF)}
        m.update(consts)
        in_maps.append(m)

    from concourse.bass_utils import run_bass_kernel_spmd
    res = run_bass_kernel_spmd(nc, in_maps, core_ids=list(range(8)))
    outs = [res.results[b]["out"].astype(np.float32).reshape(K, C, 64, 64)
            for b in range(B)]
    return np.stack(outs, axis=0)


if __name__ == "__main__":
    build_nc()
    print("built ok")


# revision 3
# speedup vs baseline: 1.0336x; 1.0336x over previous
"""Trainium2 Bass kernel v6 for nn_BaseTransformer (ensemble member-attention).

Data-parallel over batch B=8 across 8 cores. bf16 IO (host casts x down,
output back up). Per core, x = in_tensor[b] as [K=16, C=64, S=4096]:

  kq   = selu-ish: kqT stores selu(k)/lambda etc (lambda^2 folded in GRAM_SCALE)
  gram[c,i,j] = sum_s kq..., softmax over i (mask-then-reduce), B = E_norm + D
  mix per head-group g (8 heads, stride-8: c = 8u+g)
  out = selu(x + wout.T transformed + bout)    (lambda folds in weights)

Layouts (i = member = 2t + m2, pair tile t holds members 2t, 2t+1):
  xb[t]   [128=(m2,c'), S]                       resident bf16
  kqT blk [128=s, 2048], col = 1024*half + 16*h + i     (h = head, natural)
  vT blk  [128=s, 1024], col = 128*g + 16*u + i         (head c = 8u+g)
  gram_ps [p = 16u + j, f = 16u' + i] per g   (mask kills u!=u')
  rho_v(i,u) = 16u + i ; rho_T(j,u) = 16*(j//2) + 8*(j%2) + u
  vg blk  [128=rho_v, 1024], col = 128*g + s_in          (via XBAR)
  TT blk  [128=s, 1024], col = 128*jt + rho_T            (mix out copy)
  T_pair  [128=(16g+8j2+u), 4096], col = 512*jt + 128*blkw + s_in  (via XBAR)
  out psum rows = 64*j2 + o  == xb pair rows  -> lamI residual matmul
"""

import sys

if "/opt/trn_rl_repo" not in sys.path:
    sys.path.insert(0, "/opt/trn_rl_repo")

import numpy as np
import ml_dtypes

import concourse.bass as bass
import concourse.bacc as bacc
import concourse.mybir as mybir
import concourse.tile as tile

F32 = mybir.dt.float32
BF16 = mybir.dt.bfloat16
BF = ml_dtypes.bfloat16

K, C, S = 16, 64, 4096
NB = 32          # 128-col spatial blocks
SC = 128

ALPHA = 1.6732632423543772
LAMBDA = 1.0507009873554805
LN_ALPHA = float(np.log(ALPHA))
LN_LAMBDA_ALPHA = float(np.log(LAMBDA * ALPHA))
GRAM_SCALE = float(LAMBDA * LAMBDA / 64.0)


def host_constants(w_value, w_key, w_query, w_out, b_out):
    """Replicated device inputs (host-side weight shuffles + folds)."""
    c = {}
    # wkqv2 [128, 384]: rows (m2, c'); cols 192*m2 + [k 0:64 | q 64:128 | v 128:192]
    # k/q cols: head h natural. v cols: position 8*(h%8) + (h//8)  (= 8g+u)
    # cols: [k(m2=0) | k(m2=1) | q(m2=0) | q(m2=1) | v(m2=0) | v(m2=1)]
    # kq head order natural; v head at position 8*(h%8) + h//8 (= 8g+u)
    wk, wq, wv = w_key, w_query, w_value          # [heads, c_in]
    vperm = np.zeros(64, np.int64)
    for h in range(64):
        vperm[8 * (h % 8) + (h // 8)] = h          # col pos 8g+u <- head h=8u+g
    vv = wv.T[:, vperm]                             # [c', 64]
    blk = np.zeros((128, 384), np.float32)
    for m2 in range(2):
        r = slice(64 * m2, 64 * m2 + 64)
        blk[r, 64 * m2:64 * m2 + 64] = wk.T
        blk[r, 128 + 64 * m2:128 + 64 * m2 + 64] = wq.T
        blk[r, 256 + 64 * m2:256 + 64 * m2 + 64] = vv
    c["wkqv2"] = blk

    # wout2 [128, 128]: rows 16g + 8j2 + u ; cols 64*j2 + o ; val = lam*wout[o, 8u+g]
    w2 = np.zeros((128, 128), np.float32)
    for g in range(8):
        for j2 in range(2):
            for u in range(8):
                w2[16 * g + 8 * j2 + u, 64 * j2:64 * j2 + 64] = \
                    LAMBDA * w_out[:, 8 * u + g]
    c["wout2"] = w2

    c["lamI"] = (LAMBDA * np.eye(128)).astype(np.float32)

    # mask on gram [p=8j+u, f=8i+u']: keep u==u'
    mask = np.zeros((128, 128), np.float32)
    for p in range(128):
        for f in range(128):
            if p % 8 == f % 8:
                mask[p, f] = 1.0
    c["maskg"] = mask

    # Pp2 [k = 8j+u, n = rho_T(j,u)]
    pp = np.zeros((128, 128), np.float32)
    for u in range(8):
        for j in range(16):
            pp[8 * j + u, 16 * (j // 2) + 8 * (j % 2) + u] = 1.0
    c["permPp"] = pp

    # P2 [k = 8i+u, m = rho_v(i,u) = 16u+i]  (second perm lhsT)
    p2 = np.zeros((128, 128), np.float32)
    for u in range(8):
        for i in range(16):
            p2[8 * i + u, 16 * u + i] = 1.0
    c["permP2"] = p2

    # D2 [rho_v(i,u), rho_T(j,u')] = delta(u,u') (delta(i,j) - 1/16)
    d2 = np.zeros((128, 128), np.float32)
    for u in range(8):
        for i in range(16):
            for j in range(16):
                d2[16 * u + i, 16 * (j // 2) + 8 * (j % 2) + u] = \
                    (1.0 if i == j else 0.0) - 1.0 / 16.0
    c["dpat2"] = d2

    # exp2 bias: rows (j2, o): bout[o] + ln(lam*alpha);  relu2 bias: lam*bout[o]
    b1 = np.concatenate([b_out, b_out]).astype(np.float32) + LN_LAMBDA_ALPHA
    c["bias_e"] = b1.reshape(128, 1)
    b2 = LAMBDA * np.concatenate([b_out, b_out]).astype(np.float32)
    c["bias_r"] = b2.reshape(128, 1)
    return {k2: np.ascontiguousarray(v) for k2, v in c.items()}


def build_nc():
    nc = bacc.Bacc("TRN2", target_bir_lowering=False, debug=False)

    x_d = nc.dram_tensor("x", [K, C, S], BF16, kind="ExternalInput")
    wkqv_d = nc.dram_tensor("wkqv2", [128, 384], F32, kind="ExternalInput")
    wout_d = nc.dram_tensor("wout2", [128, 128], F32, kind="ExternalInput")
    lamI_d = nc.dram_tensor("lamI", [128, 128], F32, kind="ExternalInput")
    mask_d = nc.dram_tensor("maskg", [128, 128], F32, kind="ExternalInput")
    pp_d = nc.dram_tensor("permPp", [128, 128], F32, kind="ExternalInput")
    p2_d = nc.dram_tensor("permP2", [128, 128], F32, kind="ExternalInput")
    d2_d = nc.dram_tensor("dpat2", [128, 128], F32, kind="ExternalInput")
    be_d = nc.dram_tensor("bias_e", [128, 1], F32, kind="ExternalInput")
    br_d = nc.dram_tensor("bias_r", [128, 1], F32, kind="ExternalInput")
    out_d = nc.dram_tensor("out", [K, C, S], BF16, kind="ExternalOutput")

    with tile.TileContext(nc) as tc:
        with (
            tc.tile_pool(name="persist", bufs=1) as persist,
            tc.tile_pool(name="xpool", bufs=1) as xpool,
            tc.tile_pool(name="vtpool", bufs=1) as vtpool,
        ):
            # ---------- constants ----------
            def load_const(name, dram, shape, cast_bf16=False):
                t = persist.tile(shape, F32, tag=name + "f")
                nc.sync.dma_start(out=t, in_=dram[:, :])
                if not cast_bf16:
                    return t
                tb = persist.tile(shape, BF16, tag=name)
                nc.gpsimd.tensor_copy(tb, t)
                return tb

            wkqv_sb = load_const("wkqv", wkqv_d, [128, 384], cast_bf16=True)
            wout_sb = load_const("wout", wout_d, [128, 128], cast_bf16=True)
            lamI_sb = load_const("lamI", lamI_d, [128, 128], cast_bf16=True)
            mask_sb = load_const("mask", mask_d, [128, 128])
            pp_sb = load_const("pp", pp_d, [128, 128], cast_bf16=True)
            p2_sb = load_const("p2", p2_d, [128, 128], cast_bf16=True)
            d2_sb = load_const("d2", d2_d, [128, 128])
            be_sb = load_const("be", be_d, [128, 1])
            br_sb = load_const("br", br_d, [128, 1])
            lna_sb = persist.tile([128, 1], F32, tag="lna")
            nc.vector.memset(lna_sb, LN_ALPHA)

            # ---------- x resident: xb[t] = members (2t, 2t+1) ----------
            x2 = x_d.rearrange("(t m2) c s -> t (m2 c) s", t=8, m2=2)
            x_sb = []
            for t in range(8):
                xb = xpool.tile([128, S], BF16, tag=f"x{t}")
                nc.sync.dma_start(out=xb, in_=x2[t])
                x_sb.append(xb)

            # vT blocks resident until consumed by XBAR in ph2 (freed never;
            # one [128, 1024] per block -> 32 * 2KB/part = 64KB/part)
            vt_sb = []
            for b in range(NB):
                vt_sb.append(vtpool.tile([128, 1024], BF16, tag=f"vt{b}",
                                         name=f"vt{b}"))

            bigB = [persist.tile([128, 128], BF16, tag=f"bigB{g}",
                                 name=f"bigB{g}") for g in range(8)]

            # ======================= PHASE 1 =======================
            with (
                tc.tile_pool(name="p1sc", bufs=3) as p1sc,
                tc.tile_pool(name="gramps", bufs=1, space="PSUM") as gramps,
            ):
                gram_ps = [gramps.tile([128, 512], F32, tag=f"gram{gb}",
                                       name=f"gram{gb}") for gb in range(2)]

                kqvps_cm = tc.tile_pool(name="kqvps", bufs=3, space="PSUM")
                kqvps = kqvps_cm.__enter__()
                kqT_hist = {}
                for blk in range(NB):
                    sl = slice(SC * blk, SC * (blk + 1))
                    e2 = p1sc.tile([128, 2048], BF16, tag="e2")
                    r2 = p1sc.tile([128, 2048], BF16, tag="r2")
                    for (t0, nt) in ((0, 2), (2, 2), (4, 2), (6, 2)):
                        ps = kqvps.tile([128, 1024], F32, tag="kqv")
                        for tb in range(nt):
                            nc.tensor.matmul(
                                ps[:, 512 * tb: 512 * tb + 384],
                                x_sb[t0 + tb][:, sl], wkqv_sb,
                                start=True, stop=True)
                        # psum cols per pair: [k0 k1 q0 q1 | v0 v1]
                        kq_src = ps.rearrange(
                            "p (tb r) -> p tb r", tb=2, r=512
                        )[:, 0:nt, 0:256].rearrange(
                            "p tb (half mh) -> p tb half mh", half=2, mh=128)
                        # dst kqT-layout col = 1024*half + 128*t + mh
                        def shaped(dst):
                            v = dst.rearrange(
                                "p (half t mh) -> p t half mh",
                                half=2, t=8, mh=128)
                            return v[:, t0:t0 + nt]     # [p, tb, half, mh]
                        nc.scalar.activation(
                            out=shaped(e2), in_=kq_src,
                            func=mybir.ActivationFunctionType.Exp,
                            bias=lna_sb[:, 0:1])
                        nc.vector.tensor_scalar(
                            out=shaped(r2), in0=kq_src,
                            scalar1=0.0, scalar2=ALPHA,
                            op0=mybir.AluOpType.max,
                            op1=mybir.AluOpType.add)
                        # v cols [256:384]: (tb, m2: 64, hv: 1 x64)
                        # dst vT col = 16*hv + i,  i = 2t + m2
                        v_src = ps.rearrange(
                            "p (tb r) -> p tb r", tb=2, r=512
                        )[:, 0:nt, 256:384].rearrange(
                            "p tb (m2 hv) -> p tb m2 hv", m2=2, hv=64)
                        v_dst = vt_sb[blk].rearrange(
                            "p (hv i) -> p hv i", hv=64, i=16).rearrange(
                            "p hv (t m2) -> p t m2 hv",
                            t=8, m2=2)[:, t0:t0 + nt]
                        if t0 < 6:
                            nc.scalar.copy(v_dst, v_src)
                        else:
                            nc.vector.tensor_copy(v_dst, v_src)
                    # mq = min(e2, relu+a) [Pool]; kqT = mq - a = selu/lam [4x]
                    mq = p1sc.tile([128, 2048], BF16, tag="mq")
                    nc.vector.tensor_tensor(
                        out=mq, in0=e2, in1=r2, op=mybir.AluOpType.min)
                    kqT = p1sc.tile([128, 2048], BF16, tag="kqT")
                    nc.vector.tensor_scalar(
                        out=kqT, in0=mq, scalar1=ALPHA, scalar2=None,
                        op0=mybir.AluOpType.subtract)
                    # gram software-pipelined one block behind (PE never
                    # waits on this block's selu chain)
                    kqT_hist[blk] = kqT
                    def emit_gram(b):
                        kv = kqT_hist.pop(b).rearrange(
                            "p (f e) -> p e f", f=256, e=8)
                        for g in range(8):
                            nc.tensor.matmul(
                                gram_ps[g // 4][:, 128 * (g % 4):
                                                128 * (g % 4 + 1)],
                                kv[:, g, 128:256], kv[:, g, 0:128],
                                start=(b == 0), stop=(b == NB - 1))
                    if blk > 0:
                        emit_gram(blk - 1)
                    if blk == NB - 1:
                        emit_gram(blk)

                # ---------- softmax + bigB ----------
                kqvps_cm.__exit__(None, None, None)
                with tc.tile_pool(name="smps", bufs=1, space="PSUM") as smps:
                    for g in range(8):
                        gp = gram_ps[g // 4][:, 128 * (g % 4): 128 * (g % 4 + 1)]
                        E = p1sc.tile([128, 128], F32, tag="E")
                        nc.scalar.activation(
                            out=E, in_=gp,
                            func=mybir.ActivationFunctionType.Exp,
                            bias=0.0, scale=GRAM_SCALE)
                        Em = p1sc.tile([128, 128], F32, tag="Em")
                        nc.vector.tensor_tensor(
                            out=Em, in0=E, in1=mask_sb,
                            op=mybir.AluOpType.mult)
                        Ss = p1sc.tile([128, 1], F32, tag="Ss")
                        nc.vector.tensor_reduce(
                            out=Ss, in_=Em, axis=mybir.AxisListType.X,
                            op=mybir.AluOpType.add)
                        R2 = p1sc.tile([128, 1], F32, tag="R2")
                        nc.vector.reciprocal(out=R2, in_=Ss)
                        En = p1sc.tile([128, 128], BF16, tag="En")
                        nc.vector.tensor_scalar(
                            out=En, in0=Em, scalar1=R2[:, 0:1], scalar2=None,
                            op0=mybir.AluOpType.mult)
                        # c[8i+u, rho_T(j,u)] = En[8j+u, 8i+u]
                        cps = smps.tile([128, 128], F32, tag="cps")
                        nc.tensor.matmul(cps, En, pp_sb, start=True, stop=True)
                        csb = p1sc.tile([128, 128], BF16, tag="csb")
                        nc.vector.tensor_copy(csb, cps)
                        # b[rho_v(i,u), rho_T(j,u)] = c[8i+u, .] ; + D2
                        bps = smps.tile([128, 128], F32, tag="bps")
                        nc.tensor.matmul(bps, p2_sb, csb, start=True, stop=True)
                        nc.vector.scalar_tensor_tensor(
                            out=bigB[g], in0=bps, scalar=1.0, in1=d2_sb,
                            op0=mybir.AluOpType.mult, op1=mybir.AluOpType.add)

            # ======================= PHASE 2 =======================
            with (
                tc.tile_pool(name="vgp", bufs=4) as vgp,
                tc.tile_pool(name="ttp", bufs=4) as ttp,
                tc.tile_pool(name="tpp", bufs=2) as tpp,
                tc.tile_pool(name="osb", bufs=2) as osbp,
                tc.tile_pool(name="p2sc", bufs=3) as p2sc,
                tc.tile_pool(name="mixps", bufs=2, space="PSUM") as mixps,
                tc.tile_pool(name="ops", bufs=2, space="PSUM") as ops,
            ):
                ov_d = out_d.rearrange(
                    "(jt j2) c (ch s) -> ch (j2 c) jt s", jt=8, j2=2, ch=8)
                tp_hist = {}

                def emit_out(ch):
                    tp = tp_hist.pop(ch)
                    slc = slice(512 * ch, 512 * (ch + 1))
                    ost = osbp.tile([128, 4096], BF16, tag="ost")
                    for th in range(4):
                        po = ops.tile([128, 1024], F32, tag="ops")
                        for sub in range(2):
                            jt = 2 * th + sub
                            pslice = po[:, 512 * sub: 512 * (sub + 1)]
                            nc.tensor.matmul(
                                pslice, wout_sb,
                                tp[:, 512 * jt: 512 * (jt + 1)],
                                start=True, stop=False)
                            nc.tensor.matmul(
                                pslice, lamI_sb, x_sb[jt][:, slc],
                                start=False, stop=True)
                        e2o = p2sc.tile([128, 1024], BF16, tag="e2o")
                        nc.scalar.activation(
                            out=e2o, in_=po,
                            func=mybir.ActivationFunctionType.Exp,
                            bias=be_sb[:, 0:1], scale=float(1.0 / LAMBDA))
                        r3 = p2sc.tile([128, 1024], BF16, tag="r3")
                        nc.scalar.activation(
                            out=r3, in_=po,
                            func=mybir.ActivationFunctionType.Relu,
                            bias=br_sb[:, 0:1])
                        e2m = p2sc.tile([128, 1024], BF16, tag="e2m")
                        nc.vector.tensor_scalar(
                            out=e2m, in0=e2o,
                            scalar1=float(LAMBDA * ALPHA), scalar2=None,
                            op0=mybir.AluOpType.subtract)
                        nc.vector.tensor_tensor(
                            out=ost[:, 1024 * th: 1024 * (th + 1)],
                            in0=e2m, in1=r3, op=mybir.AluOpType.min)
                    ov = ost.rearrange("p (jt s) -> p jt s", jt=8, s=512)
                    nc.sync.dma_start(out=ov_d[ch], in_=ov)

                for ch in range(8):             # 512-col chunks
                    tp = tpp.tile([128, 4096], BF16, tag="tpair")
                    tp_hist[ch] = tp
                    for blkw in range(4):
                        blk = 4 * ch + blkw
                        # v regroup: vT [s, (g, rho_v)] -> vg [rho_v, (g, s)]
                        vg = vgp.tile([128, 1024], BF16, tag="vg")
                        vgv = vg.rearrange("p (g s) -> p g s", g=8, s=128)
                        nc.sync.dma_start(out=vgv, in_=vt_sb[blk],
                                          transpose=True)
                        # mix (v stationary): psum [s, rho_T] per g
                        mps = mixps.tile([128, 1024], F32, tag="mix")
                        for g in range(8):
                            nc.tensor.matmul(
                                mps[:, 128 * g: 128 * (g + 1)],
                                vgv[:, g], bigB[g], start=True, stop=True)
                        # copy -> TT col = 128*jt + 16*g + r  (rho_T=(jt,j2,u))
                        tt = ttp.tile([128, 1024], BF16, tag="tt")
                        tt_dst = tt.rearrange(
                            "p (jt g r) -> p g jt r", jt=8, g=8, r=16)
                        mps_src = mps.rearrange(
                            "p (g jt r) -> p g jt r", g=8, jt=8, r=16)
                        nc.vector.tensor_copy(tt_dst, mps_src)
                        # T regroup: TT [s, (jt, rho)] -> T_pair [rho, (jt, s)]
                        tpv = tp.rearrange("p (jt w s) -> p jt w s",
                                           jt=8, w=4, s=128)[:, :, blkw]
                        nc.sync.dma_start(out=tpv, in_=tt, transpose=True)
                    if ch > 0:
                        emit_out(ch - 1)
                    if ch == 7:
                        emit_out(ch)
    nc.compile()
    return nc


_NC_CACHE = None


def _get_nc():
    global _NC_CACHE
    if _NC_CACHE is None:
        _NC_CACHE = build_nc()
    return _NC_CACHE


def kernel(in_tensor, w_value, w_key, w_query, w_out, b_out, **_ignored):
    in_tensor = np.asarray(in_tensor, dtype=np.float32)
    consts = host_constants(
        np.asarray(w_value, np.float32), np.asarray(w_key, np.float32),
        np.asarray(w_query, np.float32), np.asarray(w_out, np.float32),
        np.asarray(b_out, np.float32))

    B = in_tensor.shape[0]
    assert B == 8
    nc = _get_nc()
    in_maps = []
    for b in range(B):
        m = {"x": np.ascontiguousarray(
            in_tensor[b].reshape(K, C, S)).astype(BF)}
        m.update(consts)
        in_maps.append(m)

    from concourse.bass_utils import run_bass_kernel_spmd
    res = run_bass_kernel_spmd(nc, in_maps, core_ids=list(range(8)))
    outs = [res.results[b]["out"].astype(np.float32).reshape(K, C, 64, 64)
            for b in range(B)]
    return np.stack(outs, axis=0)


if __name__ == "__main__":
    build_nc()
    print("built ok")


# revision 4
# speedup vs baseline: 1.0471x; 1.0131x over previous
"""Trainium2 Bass kernel v6 for nn_BaseTransformer (ensemble member-attention).

Data-parallel over batch B=8 across 8 cores. bf16 IO (host casts x down,
output back up). Per core, x = in_tensor[b] as [K=16, C=64, S=4096]:

  kq   = selu-ish: kqT stores selu(k)/lambda etc (lambda^2 folded in GRAM_SCALE)
  gram[c,i,j] = sum_s kq..., softmax over i (mask-then-reduce), B = E_norm + D
  mix per head-group g (8 heads, stride-8: c = 8u+g)
  out = selu(x + wout.T transformed + bout)    (lambda folds in weights)

Layouts (i = member = 2t + m2, pair tile t holds members 2t, 2t+1):
  xb[t]   [128=(m2,c'), S]                       resident bf16
  kqT blk [128=s, 2048], col = 1024*half + 64*m + h     (h = head, natural)
  vT blk  [128=s, 1024], col = 16*(8g+u) + i            (head c = 8u+g)
  gram_ps [p = 16u + j, f = 16u' + i] per g   (mask kills u!=u')
  rho_v(i,u) = 16u + i ; rho_T(j,u) = 16*(j//2) + 8*(j%2) + u
  vg blk  [128=rho_v, 1024], col = 128*g + s_in          (via XBAR)
  TT blk  [128=s, 1024], col = 128*jt + rho_T            (mix out copy)
  T_pair  [128=(16g+8j2+u), 4096], col = 512*jt + 128*blkw + s_in  (via XBAR)
  out psum rows = 64*j2 + o  == xb pair rows  -> lamI residual matmul
"""

import sys

if "/opt/trn_rl_repo" not in sys.path:
    sys.path.insert(0, "/opt/trn_rl_repo")

import numpy as np
import ml_dtypes

import concourse.bass as bass
import concourse.bacc as bacc
import concourse.mybir as mybir
import concourse.tile as tile

F32 = mybir.dt.float32
BF16 = mybir.dt.bfloat16
BF = ml_dtypes.bfloat16

K, C, S = 16, 64, 4096
NB = 32          # 128-col spatial blocks
SC = 128

ALPHA = 1.6732632423543772
LAMBDA = 1.0507009873554805
LN_ALPHA = float(np.log(ALPHA))
LN_LAMBDA_ALPHA = float(np.log(LAMBDA * ALPHA))
GRAM_SCALE = float(LAMBDA * LAMBDA / 64.0)


def host_constants(w_value, w_key, w_query, w_out, b_out):
    """Replicated device inputs (host-side weight shuffles + folds)."""
    c = {}
    # wkqv2 [128, 384]: rows (m2, c'); cols 192*m2 + [k 0:64 | q 64:128 | v 128:192]
    # k/q cols: head h natural. v cols: position 8*(h%8) + (h//8)  (= 8g+u)
    # cols: [k(m2=0) | k(m2=1) | q(m2=0) | q(m2=1) | v(m2=0) | v(m2=1)]
    # kq head order natural; v head at position 8*(h%8) + h//8 (= 8g+u)
    wk, wq, wv = w_key, w_query, w_value          # [heads, c_in]
    vperm = np.zeros(64, np.int64)
    for h in range(64):
        vperm[8 * (h % 8) + (h // 8)] = h          # col pos 8g+u <- head h=8u+g
    vv = wv.T[:, vperm]                             # [c', 64]
    blk = np.zeros((128, 384), np.float32)
    for m2 in range(2):
        r = slice(64 * m2, 64 * m2 + 64)
        blk[r, 64 * m2:64 * m2 + 64] = wk.T
        blk[r, 128 + 64 * m2:128 + 64 * m2 + 64] = wq.T
        blk[r, 256 + 64 * m2:256 + 64 * m2 + 64] = vv
    c["wkqv2"] = blk

    # wout2 [128, 128]: rows 16g + 8j2 + u ; cols 64*j2 + o ; val = lam*wout[o, 8u+g]
    w2 = np.zeros((128, 128), np.float32)
    for g in range(8):
        for j2 in range(2):
            for u in range(8):
                w2[16 * g + 8 * j2 + u, 64 * j2:64 * j2 + 64] = \
                    LAMBDA * w_out[:, 8 * u + g]
    c["wout2"] = w2

    c["lamI"] = (LAMBDA * np.eye(128)).astype(np.float32)

    # mask on gram [p=8j+u, f=8i+u']: keep u==u'
    mask = np.zeros((128, 128), np.float32)
    for p in range(128):
        for f in range(128):
            if p % 8 == f % 8:
                mask[p, f] = 1.0
    c["maskg"] = mask

    # Pp2 [k = 8j+u, n = rho_T(j,u)]
    pp = np.zeros((128, 128), np.float32)
    for u in range(8):
        for j in range(16):
            pp[8 * j + u, 16 * (j // 2) + 8 * (j % 2) + u] = 1.0
    c["permPp"] = pp

    # P2 [k = 8i+u, m = rho_v(i,u) = 16u+i]  (second perm lhsT)
    p2 = np.zeros((128, 128), np.float32)
    for u in range(8):
        for i in range(16):
            p2[8 * i + u, 16 * u + i] = 1.0
    c["permP2"] = p2

    # D2 [rho_v(i,u), rho_T(j,u')] = delta(u,u') (delta(i,j) - 1/16)
    d2 = np.zeros((128, 128), np.float32)
    for u in range(8):
        for i in range(16):
            for j in range(16):
                d2[16 * u + i, 16 * (j // 2) + 8 * (j % 2) + u] = \
                    (1.0 if i == j else 0.0) - 1.0 / 16.0
    c["dpat2"] = d2

    # exp2 bias: rows (j2, o): bout[o] + ln(lam*alpha);  relu2 bias: lam*bout[o]
    b1 = np.concatenate([b_out, b_out]).astype(np.float32) + LN_LAMBDA_ALPHA
    c["bias_e"] = b1.reshape(128, 1)
    b2 = LAMBDA * np.concatenate([b_out, b_out]).astype(np.float32)
    c["bias_r"] = b2.reshape(128, 1)
    return {k2: np.ascontiguousarray(v) for k2, v in c.items()}


def build_nc():
    nc = bacc.Bacc("TRN2", target_bir_lowering=False, debug=False)

    x_d = nc.dram_tensor("x", [K, C, S], BF16, kind="ExternalInput")
    wkqv_d = nc.dram_tensor("wkqv2", [128, 384], F32, kind="ExternalInput")
    wout_d = nc.dram_tensor("wout2", [128, 128], F32, kind="ExternalInput")
    lamI_d = nc.dram_tensor("lamI", [128, 128], F32, kind="ExternalInput")
    mask_d = nc.dram_tensor("maskg", [128, 128], F32, kind="ExternalInput")
    pp_d = nc.dram_tensor("permPp", [128, 128], F32, kind="ExternalInput")
    p2_d = nc.dram_tensor("permP2", [128, 128], F32, kind="ExternalInput")
    d2_d = nc.dram_tensor("dpat2", [128, 128], F32, kind="ExternalInput")
    be_d = nc.dram_tensor("bias_e", [128, 1], F32, kind="ExternalInput")
    br_d = nc.dram_tensor("bias_r", [128, 1], F32, kind="ExternalInput")
    out_d = nc.dram_tensor("out", [K, C, S], BF16, kind="ExternalOutput")

    with tile.TileContext(nc) as tc:
        with (
            tc.tile_pool(name="persist", bufs=1) as persist,
            tc.tile_pool(name="xpool", bufs=1) as xpool,
            tc.tile_pool(name="vtpool", bufs=1) as vtpool,
        ):
            # ---------- constants ----------
            def load_const(name, dram, shape, cast_bf16=False):
                t = persist.tile(shape, F32, tag=name + "f")
                nc.sync.dma_start(out=t, in_=dram[:, :])
                if not cast_bf16:
                    return t
                tb = persist.tile(shape, BF16, tag=name)
                nc.gpsimd.tensor_copy(tb, t)
                return tb

            wkqv_sb = load_const("wkqv", wkqv_d, [128, 384], cast_bf16=True)
            wout_sb = load_const("wout", wout_d, [128, 128], cast_bf16=True)
            lamI_sb = load_const("lamI", lamI_d, [128, 128], cast_bf16=True)
            mask_sb = load_const("mask", mask_d, [128, 128])
            pp_sb = load_const("pp", pp_d, [128, 128], cast_bf16=True)
            p2_sb = load_const("p2", p2_d, [128, 128], cast_bf16=True)
            d2_sb = load_const("d2", d2_d, [128, 128])
            be_sb = load_const("be", be_d, [128, 1])
            br_sb = load_const("br", br_d, [128, 1])
            lna_sb = persist.tile([128, 1], F32, tag="lna")
            nc.vector.memset(lna_sb, LN_ALPHA)

            # ---------- x resident: xb[t] = members (2t, 2t+1) ----------
            x2 = x_d.rearrange("(t m2) c s -> t (m2 c) s", t=8, m2=2)
            x_sb = []
            for t in range(8):
                xb = xpool.tile([128, S], BF16, tag=f"x{t}")
                nc.sync.dma_start(out=xb, in_=x2[t])
                x_sb.append(xb)

            # vT blocks resident until consumed by XBAR in ph2 (freed never;
            # one [128, 1024] per block -> 32 * 2KB/part = 64KB/part)
            vt_sb = []
            for b in range(NB):
                vt_sb.append(vtpool.tile([128, 1024], BF16, tag=f"vt{b}",
                                         name=f"vt{b}"))

            bigB = [persist.tile([128, 128], BF16, tag=f"bigB{g}",
                                 name=f"bigB{g}") for g in range(8)]

            # ======================= PHASE 1 =======================
            with (
                tc.tile_pool(name="p1sc", bufs=3) as p1sc,
                tc.tile_pool(name="gramps", bufs=1, space="PSUM") as gramps,
            ):
                gram_ps = [gramps.tile([128, 512], F32, tag=f"gram{gb}",
                                       name=f"gram{gb}") for gb in range(2)]

                kqvps_cm = tc.tile_pool(name="kqvps", bufs=3, space="PSUM")
                kqvps = kqvps_cm.__enter__()
                kqT_hist = {}
                for blk in range(NB):
                    sl = slice(SC * blk, SC * (blk + 1))
                    e2 = p1sc.tile([128, 2048], BF16, tag="e2")
                    r2 = p1sc.tile([128, 2048], BF16, tag="r2")
                    for (t0, nt) in ((0, 2), (2, 2), (4, 2), (6, 2)):
                        ps = kqvps.tile([128, 1024], F32, tag="kqv")
                        for tb in range(nt):
                            nc.tensor.matmul(
                                ps[:, 512 * tb: 512 * tb + 384],
                                x_sb[t0 + tb][:, sl], wkqv_sb,
                                start=True, stop=True)
                        # psum cols per pair: [k0 k1 q0 q1 | v0 v1]
                        kq_src = ps.rearrange(
                            "p (tb r) -> p tb r", tb=2, r=512
                        )[:, 0:nt, 0:256].rearrange(
                            "p tb (half mh) -> p tb half mh", half=2, mh=128)
                        # dst kqT-layout col = 1024*half + 128*t + mh
                        def shaped(dst):
                            v = dst.rearrange(
                                "p (half t mh) -> p t half mh",
                                half=2, t=8, mh=128)
                            return v[:, t0:t0 + nt]     # [p, tb, half, mh]
                        nc.scalar.activation(
                            out=shaped(e2), in_=kq_src,
                            func=mybir.ActivationFunctionType.Exp,
                            bias=lna_sb[:, 0:1])
                        nc.vector.tensor_scalar(
                            out=shaped(r2), in0=kq_src,
                            scalar1=0.0, scalar2=ALPHA,
                            op0=mybir.AluOpType.max,
                            op1=mybir.AluOpType.add)
                        # v cols [256:384]: (tb, m2: 64, hv: 1 x64)
                        # dst vT col = 16*hv + i,  i = 2t + m2
                        v_src = ps.rearrange(
                            "p (tb r) -> p tb r", tb=2, r=512
                        )[:, 0:nt, 256:384].rearrange(
                            "p tb (m2 hv) -> p tb m2 hv", m2=2, hv=64)
                        v_dst = vt_sb[blk].rearrange(
                            "p (hv i) -> p hv i", hv=64, i=16).rearrange(
                            "p hv (t m2) -> p t m2 hv",
                            t=8, m2=2)[:, t0:t0 + nt]
                        if t0 < 6:
                            nc.scalar.copy(v_dst, v_src)
                        else:
                            nc.vector.tensor_copy(v_dst, v_src)
                    # mq = min(e2, relu+a) [Pool]; kqT = mq - a = selu/lam [4x]
                    mq = p1sc.tile([128, 2048], BF16, tag="mq")
                    nc.vector.tensor_tensor(
                        out=mq, in0=e2, in1=r2, op=mybir.AluOpType.min)
                    kqT = p1sc.tile([128, 2048], BF16, tag="kqT")
                    nc.vector.tensor_scalar(
                        out=kqT, in0=mq, scalar1=ALPHA, scalar2=None,
                        op0=mybir.AluOpType.subtract)
                    # gram software-pipelined one block behind (PE never
                    # waits on this block's selu chain)
                    kqT_hist[blk] = kqT
                    def emit_gram(b):
                        kv = kqT_hist.pop(b).rearrange(
                            "p (f e) -> p e f", f=256, e=8)
                        for g in range(8):
                            nc.tensor.matmul(
                                gram_ps[g // 4][:, 128 * (g % 4):
                                                128 * (g % 4 + 1)],
                                kv[:, g, 128:256], kv[:, g, 0:128],
                                start=(b == 0), stop=(b == NB - 1))
                    if blk > 0:
                        emit_gram(blk - 1)
                    if blk == NB - 1:
                        emit_gram(blk)

                # ---------- softmax + bigB ----------
                kqvps_cm.__exit__(None, None, None)
                with tc.tile_pool(name="smps", bufs=1, space="PSUM") as smps:
                    for g in range(8):
                        gp = gram_ps[g // 4][:, 128 * (g % 4): 128 * (g % 4 + 1)]
                        E = p1sc.tile([128, 128], F32, tag="E")
                        nc.scalar.activation(
                            out=E, in_=gp,
                            func=mybir.ActivationFunctionType.Exp,
                            bias=0.0, scale=GRAM_SCALE)
                        Em = p1sc.tile([128, 128], F32, tag="Em")
                        nc.vector.tensor_tensor(
                            out=Em, in0=E, in1=mask_sb,
                            op=mybir.AluOpType.mult)
                        Ss = p1sc.tile([128, 1], F32, tag="Ss")
                        nc.vector.tensor_reduce(
                            out=Ss, in_=Em, axis=mybir.AxisListType.X,
                            op=mybir.AluOpType.add)
                        R2 = p1sc.tile([128, 1], F32, tag="R2")
                        nc.vector.reciprocal(out=R2, in_=Ss)
                        En = p1sc.tile([128, 128], BF16, tag="En")
                        nc.vector.tensor_scalar(
                            out=En, in0=Em, scalar1=R2[:, 0:1], scalar2=None,
                            op0=mybir.AluOpType.mult)
                        # c[8i+u, rho_T(j,u)] = En[8j+u, 8i+u]
                        cps = smps.tile([128, 128], F32, tag="cps")
                        nc.tensor.matmul(cps, En, pp_sb, start=True, stop=True)
                        csb = p1sc.tile([128, 128], BF16, tag="csb")
                        nc.vector.tensor_copy(csb, cps)
                        # b[rho_v(i,u), rho_T(j,u)] = c[8i+u, .] ; + D2
                        bps = smps.tile([128, 128], F32, tag="bps")
                        nc.tensor.matmul(bps, p2_sb, csb, start=True, stop=True)
                        nc.vector.scalar_tensor_tensor(
                            out=bigB[g], in0=bps, scalar=1.0, in1=d2_sb,
                            op0=mybir.AluOpType.mult, op1=mybir.AluOpType.add)

            # ======================= PHASE 2 =======================
            with (
                tc.tile_pool(name="vgp", bufs=4) as vgp,
                tc.tile_pool(name="ttp", bufs=4) as ttp,
                tc.tile_pool(name="tpp", bufs=2) as tpp,
                tc.tile_pool(name="osb", bufs=2) as osbp,
                tc.tile_pool(name="p2sc", bufs=3) as p2sc,
                tc.tile_pool(name="mixps", bufs=2, space="PSUM") as mixps,
                tc.tile_pool(name="ops", bufs=2, space="PSUM") as ops,
            ):
                ov_d = out_d.rearrange(
                    "(jt j2) c (ch s) -> ch (j2 c) jt s", jt=8, j2=2, ch=8)
                tp_hist = {}

                def emit_out(ch):
                    tp = tp_hist.pop(ch)
                    slc = slice(512 * ch, 512 * (ch + 1))
                    ost = osbp.tile([128, 4096], BF16, tag="ost")
                    for th in range(4):
                        po = ops.tile([128, 1024], F32, tag="ops")
                        for sub in range(2):
                            jt = 2 * th + sub
                            pslice = po[:, 512 * sub: 512 * (sub + 1)]
                            nc.tensor.matmul(
                                pslice, wout_sb,
                                tp[:, 512 * jt: 512 * (jt + 1)],
                                start=True, stop=False)
                            nc.tensor.matmul(
                                pslice, lamI_sb, x_sb[jt][:, slc],
                                start=False, stop=True)
                        e2o = p2sc.tile([128, 1024], BF16, tag="e2o")
                        nc.scalar.activation(
                            out=e2o, in_=po,
                            func=mybir.ActivationFunctionType.Exp,
                            bias=be_sb[:, 0:1], scale=float(1.0 / LAMBDA))
                        r3 = p2sc.tile([128, 1024], BF16, tag="r3")
                        nc.scalar.activation(
                            out=r3, in_=po,
                            func=mybir.ActivationFunctionType.Relu,
                            bias=br_sb[:, 0:1])
                        e2m = p2sc.tile([128, 1024], BF16, tag="e2m")
                        nc.vector.tensor_scalar(
                            out=e2m, in0=e2o,
                            scalar1=float(LAMBDA * ALPHA), scalar2=None,
                            op0=mybir.AluOpType.subtract)
                        nc.vector.tensor_tensor(
                            out=ost[:, 1024 * th: 1024 * (th + 1)],
                            in0=e2m, in1=r3, op=mybir.AluOpType.min)
                    ov = ost.rearrange("p (jt s) -> p jt s", jt=8, s=512)
                    nc.sync.dma_start(out=ov_d[ch], in_=ov)

                for ch in range(8):             # 512-col chunks
                    tp = tpp.tile([128, 4096], BF16, tag="tpair")
                    tp_hist[ch] = tp
                    for blkw in range(4):
                        blk = 4 * ch + blkw
                        # v regroup: vT [s, (g, rho_v)] -> vg [rho_v, (g, s)]
                        vg = vgp.tile([128, 1024], BF16, tag="vg")
                        vgv = vg.rearrange("p (g s) -> p g s", g=8, s=128)
                        nc.sync.dma_start(out=vgv, in_=vt_sb[blk],
                                          transpose=True)
                        # mix (v stationary): psum [s, rho_T] per g
                        mps = mixps.tile([128, 1024], F32, tag="mix")
                        for g in range(8):
                            nc.tensor.matmul(
                                mps[:, 128 * g: 128 * (g + 1)],
                                vgv[:, g], bigB[g], start=True, stop=True)
                        # copy -> TT col = 128*jt + 16*g + r  (rho_T=(jt,j2,u))
                        tt = ttp.tile([128, 1024], BF16, tag="tt")
                        tt_dst = tt.rearrange(
                            "p (jt g r) -> p g jt r", jt=8, g=8, r=16)
                        mps_src = mps.rearrange(
                            "p (g jt r) -> p g jt r", g=8, jt=8, r=16)
                        nc.vector.tensor_copy(tt_dst, mps_src)
                        # T regroup: TT [s, (jt, rho)] -> T_pair [rho, (jt, s)]
                        tpv = tp.rearrange("p (jt w s) -> p jt w s",
                                           jt=8, w=4, s=128)[:, :, blkw]
                        nc.sync.dma_start(out=tpv, in_=tt, transpose=True)
                    if ch > 0:
                        emit_out(ch - 1)
                    if ch == 7:
                        emit_out(ch)
    nc.compile()
    return nc


_NC_CACHE = None


def _get_nc():
    global _NC_CACHE
    if _NC_CACHE is None:
        _NC_CACHE = build_nc()
    return _NC_CACHE


def kernel(in_tensor, w_value, w_key, w_query, w_out, b_out, **_ignored):
    in_tensor = np.asarray(in_tensor, dtype=np.float32)
    consts = host_constants(
        np.asarray(w_value, np.float32), np.asarray(w_key, np.float32),
        np.asarray(w_query, np.float32), np.asarray(w_out, np.float32),
        np.asarray(b_out, np.float32))

    B = in_tensor.shape[0]
    assert B == 8
    nc = _get_nc()
    in_maps = []
    for b in range(B):
        m = {"x": np.ascontiguousarray(
            in_tensor[b].reshape(K, C, S)).astype(BF)}
        m.update(consts)
        in_maps.append(m)

    from concourse.bass_utils import run_bass_kernel_spmd
    res = run_bass_kernel_spmd(nc, in_maps, core_ids=list(range(8)))
    outs = [res.results[b]["out"].astype(np.float32).reshape(K, C, 64, 64)
            for b in range(B)]
    return np.stack(outs, axis=0)


if __name__ == "__main__":
    build_nc()
    print("built ok")
